# revision 1
# baseline (speedup 1.0000x reference)
"""Fused LayerNorm + multi-head attention Trainium2 kernel, 8-core SPMD.

Problem: x[4, 2048, 768] -> LN -> QKV (w_qkv[2304, 768]) -> 12-head attention
         -> out proj (w_out[768, 768] + b_out). f32 I/O, bf16 tensor-engine compute.

Sharding: core c handles batch b=c//2, query-half g=c%2 (1024 queries each).
Each core receives the FULL (rotated) sequence of its batch so K/V are computed
locally -- no collectives. The token order is rotated per-core so the core's own
query chunk is always columns [0, 1024) => identical SPMD program on all cores.

On-core layout: channel-major. LayerNorm mean/bias are folded into the QKV
matmul via two appended contraction rows; the softmax denominator is computed by
an appended ones-column in the AV matmul's stationary operand.
"""

import numpy as np
import ml_dtypes

import concourse.bass as bass
import concourse.tile as tile
from concourse import bacc, mybir
from concourse.bass_utils import run_bass_kernel_spmd

F32 = mybir.dt.float32
BF16 = mybir.dt.bfloat16
AF = mybir.ActivationFunctionType
ALU = mybir.AluOpType

DIM = 768
HEADS = 12
B, N = 4, 2048
D = 64          # head dim
NQ = 1024       # queries per core
CT = 6          # 768 / 128 channel tiles
NT = 16         # 2048 / 128 token tiles
HP = 6          # head pairs

LAST = None  # BassKernelResults of the most recent run (for test harness)
_NC = None


def build(debug=False):
    nc = bacc.Bacc("TRN2", target_bir_lowering=False, debug=False, num_devices=8)

    xT = nc.dram_tensor("xT", [DIM, N], F32, kind="ExternalInput")
    wqkvT = nc.dram_tensor("wqkvT", [DIM + 2, 3 * DIM], BF16, kind="ExternalInput")
    woutT = nc.dram_tensor("woutT", [DIM + 1, DIM], BF16, kind="ExternalInput")
    outT = nc.dram_tensor("outT", [DIM, NQ], F32, kind="ExternalOutput")
    if debug:
        d_rows = nc.dram_tensor("d_rows", [2, N], F32, kind="ExternalOutput")
        d_xtil = nc.dram_tensor("d_xtil", [128, CT, N], BF16, kind="ExternalOutput")
        d_KT = nc.dram_tensor("d_KT", [128, CT, N], BF16, kind="ExternalOutput")
        d_QT = nc.dram_tensor("d_QT", [128, CT, NQ], BF16, kind="ExternalOutput")
        d_V4 = nc.dram_tensor("d_V4", [128, NT, HEADS, D + 1], BF16, kind="ExternalOutput")
        d_AO = nc.dram_tensor("d_AO", [128, CT, NQ], BF16, kind="ExternalOutput")

    with tile.TileContext(nc) as tc:
        with (
            tc.tile_pool(name="persist", bufs=1) as P1,
            tc.tile_pool(name="work", bufs=2) as PW,
            tc.tile_pool(name="rows", bufs=2) as PR,
            tc.tile_pool(name="ps", bufs=4, space="PSUM") as PS,
        ):
            # ---- persistent SBUF tensors ----
            wq = P1.tile([128, CT, 3 * DIM], BF16)       # W'' rows 0..767
            wex = P1.tile([2, 3 * DIM], BF16)            # W'' rows 768..769
            WO = P1.tile([128, CT, DIM], BF16)           # w_out^T  (f-major tiles)
            wob = P1.tile([1, DIM], BF16)                # b_out row
            xtil = P1.tile([128, CT, N], BF16, tag="big_a")   # x~ = x^T * rstd
            xex = P1.tile([2, N], BF16)                  # x~ rows 768 (-mu*rstd), 769 (1)
            KT = P1.tile([128, CT, N], BF16)             # K^T channel-major
            QT = P1.tile([128, CT, NQ], BF16)            # Q^T channel-major
            V4 = P1.tile([128, NT, HEADS, D + 1], BF16)  # V token-major + ones col
            ones1 = P1.tile([128, 1], F32)               # f32 ones column (stats lhsT)
            ones1b = P1.tile([128, 1], BF16)             # bf16 ones column
            onesr = P1.tile([1, 128], F32)               # f32 ones row (bcast lhsT)
            onesI = P1.tile([1, NQ], BF16)               # bf16 ones row (bias rhs)
            epsc = P1.tile([1, 1], F32)

            nc.vector.memset(epsc[:], 1e-5)
            nc.vector.memset(ones1[:], 1.0)
            nc.vector.memset(ones1b[:], 1.0)
            nc.vector.memset(onesr[:], 1.0)
            nc.vector.memset(onesI[:], 1.0)
            # row 1 must stay 1.0; row 0 is overwritten with -mu*rstd in phase B
            nc.vector.memset(xex[:, :], 1.0)
            nc.vector.memset(V4[:, :, :, D : D + 1], 1.0)

            # ---- weight DMAs ----
            for ct in range(CT):
                nc.sync.dma_start(wq[:, ct, :], wqkvT[ct * 128 : (ct + 1) * 128, :])
                nc.sync.dma_start(WO[:, ct, :], woutT[ct * 128 : (ct + 1) * 128, :])
            nc.sync.dma_start(wex[:], wqkvT[DIM : DIM + 2, :])
            nc.sync.dma_start(wob[:], woutT[DIM : DIM + 1, :])

            # ---- phase A: token stats (sum, sumsq) via ones-matmul reduction ----
            # rows 0=sum, 64=sumsq, 32=mu^2 scratch; one tile per 1024-column half
            sts = [PS.tile([128, 1024], F32, tag="ps", name=f"st_{h}") for h in range(2)]
            for ct in range(CT):
                for h in range(2):
                    xin = PW.tile([128, 1024], F32, tag="xin", name=f"xinA_{ct}_{h}")
                    nc.sync.dma_start(
                        xin[:], xT[ct * 128 : (ct + 1) * 128, h * 1024 : (h + 1) * 1024]
                    )
                    xsq = PW.tile([128, 1024], BF16, tag="xsq")
                    nc.vector.tensor_tensor(xsq[:], xin[:], xin[:], ALU.mult)
                    st = sts[h]
                    for s in range(2):
                        osl = slice(s * 512, (s + 1) * 512)
                        nc.tensor.matmul(
                            st[0:1, osl], ones1[:], xin[:, osl],
                            start=(ct == 0), stop=(ct == CT - 1),
                        )
                        nc.tensor.matmul(
                            st[64:65, osl], ones1b[:], xsq[:, osl],
                            start=(ct == 0), stop=(ct == CT - 1),
                        )

            r_a = PR.tile([1, N], F32, tag="row")   # E[x^2] -> var -> rstd
            r_b = PR.tile([1, N], F32, tag="row")   # mu -> mu*rstd
            for h in range(2):
                hsl = slice(h * 1024, (h + 1) * 1024)
                nc.vector.tensor_scalar_mul(r_a[:, hsl], sts[h][64:65, :], 1.0 / DIM)
                nc.vector.tensor_scalar_mul(r_b[:, hsl], sts[h][0:1, :], 1.0 / DIM)
                nc.vector.tensor_tensor(sts[h][32:33, :], r_b[:, hsl], r_b[:, hsl], ALU.mult)
                nc.vector.tensor_tensor(r_a[:, hsl], r_a[:, hsl], sts[h][32:33, :], ALU.subtract)
            # rstd = exp(-0.5 * ln(var + eps))  (ln+exp share one ACT table set)
            nc.scalar.activation(r_a[:], r_a[:], AF.Ln, bias=epsc[:])
            nc.scalar.activation(r_a[:], r_a[:], AF.Exp, scale=-0.5)
            if debug:
                nc.sync.dma_start(d_rows[0:1, :], r_a[:])
                nc.sync.dma_start(d_rows[1:2, :], r_b[:])

            # ---- phase B: x~ = x * rstd (bf16), plus folded-LN extra rows ----
            ps_rbs = [PS.tile([128, 1024], F32, tag="ps", name=f"rb_{h}") for h in range(2)]
            for jc in range(4):
                sl = slice(jc * 512, (jc + 1) * 512)
                nc.tensor.matmul(
                    ps_rbs[jc // 2][:, (jc % 2) * 512 : (jc % 2 + 1) * 512],
                    onesr[:], r_a[:, sl],
                )
            for ct in range(CT):
                for h in range(2):
                    hsl = slice(h * 1024, (h + 1) * 1024)
                    xin = PW.tile([128, 1024], F32, tag="xin", name=f"xinB_{ct}_{h}")
                    nc.sync.dma_start(xin[:], xT[ct * 128 : (ct + 1) * 128, hsl])
                    nc.vector.tensor_tensor(
                        xtil[:, ct, hsl], xin[:], ps_rbs[h][:], ALU.mult
                    )
            nc.vector.tensor_tensor(r_b[:], r_b[:], r_a[:], ALU.mult)
            nc.vector.tensor_scalar_mul(xex[0:1, :], r_b[:], -1.0)
            if debug:
                for ct in range(CT):
                    nc.sync.dma_start(d_xtil[:, ct, :], xtil[:, ct, :])

            # ---- phase C: QKV projections ----
            # K/Q tiles are produced as incremental 512-column accumulation
            # units so single matmuls can be dribbled into the attention loop
            # as PE filler (keeps the PE dense and HAM-warm while ACT runs exp)
            class KQEmitter:
                def __init__(self, units):
                    self.units = list(units)
                    self.cur = None
                    self.ct = 0

                def _open(self):
                    which, fidx, nh, s = self.units.pop(0)
                    base = DIM if which == "K" else 0
                    self.fsl = slice(base + fidx * 128, base + (fidx + 1) * 128)
                    self.n0 = nh * 1024 + s * 512
                    self.dst = (KT if which == "K" else QT)[
                        :, fidx, self.n0 : self.n0 + 512
                    ]
                    self.acc = PS.tile(
                        [128, 512], F32, tag="ps", name=f"kq{which}_{fidx}_{nh}_{s}"
                    )
                    self.cur = which
                    self.ct = 0

                def emit(self, n):
                    while n > 0 and (self.cur is not None or self.units):
                        if self.cur is None:
                            self._open()
                        ct = self.ct
                        nsl = slice(self.n0, self.n0 + 512)
                        nc.tensor.matmul(
                            self.acc[:],
                            wq[:, ct, self.fsl] if ct < CT else wex[:, self.fsl],
                            xtil[:, ct, nsl] if ct < CT else xex[:, nsl],
                            start=(ct == 0), stop=(ct == CT),
                        )
                        self.ct += 1
                        n -= 1
                        if self.ct == CT + 1:
                            nc.vector.tensor_copy(self.dst, self.acc[:])
                            self.cur = None

                def drain(self):
                    self.emit(10 ** 9)

            def kq_units_for(fidx):
                units = [("K", fidx, nh, s) for nh in range(2) for s in range(2)]
                units += [("Q", fidx, 0, s) for s in range(2)]
                return units

            # V (token-major): out[n, f] ; lhsT = x~ tile, rhs = W'' v-columns
            for nt in range(NT):
                nsl = slice(nt * 128, (nt + 1) * 128)
                acc = PS.tile([128, 1024], F32, tag="ps")
                for ct in range(CT + 1):
                    # matmul output must not straddle a 2KB PSUM bank: 768 = 512 + 256
                    for lo, sz in ((0, 512), (512, 256)):
                        fsl = slice(2 * DIM + lo, 2 * DIM + lo + sz)
                        nc.tensor.matmul(
                            acc[:, lo : lo + sz],
                            xtil[:, ct, nsl] if ct < CT else xex[:, nsl],
                            wq[:, ct, fsl] if ct < CT else wex[:, fsl],
                            start=(ct == 0), stop=(ct == CT),
                        )
                nc.vector.tensor_copy(
                    V4[:, nt, :, 0:D],
                    acc[:, 0:DIM].rearrange("p (h d) -> p h d", h=HEADS),
                )

            # first pair's K/Q upfront
            KQEmitter(kq_units_for(0)).drain()

            # attention output, same f-major tile layout the out-proj consumes
            AO = P1.tile([128, CT, NQ], BF16, tag="AO", name="AO")

            # ---- phase D: attention, one head-pair at a time ----
            for hp in range(HP):
                emitter = KQEmitter(kq_units_for(hp + 1) if hp + 1 < HP else [])
                U0 = PS.tile([128, 1024], F32, tag="ps", name=f"U0_{hp}")
                U1 = PS.tile([128, 1024], F32, tag="ps", name=f"U1_{hp}")
                for jt in range(NT):
                    jsl = slice(jt * 128, (jt + 1) * 128)
                    for h01 in range(2):
                        psl = slice(h01 * 64, (h01 + 1) * 64)
                        sp = PS.tile([128, 1024], F32, tag="ps", name=f"sp_{hp}_{jt}_{h01}")
                        for s in range(2):
                            ssl = slice(s * 512, (s + 1) * 512)
                            nc.tensor.matmul(
                                sp[:, ssl], KT[psl, hp, jsl], QT[psl, hp, ssl],
                                start=True, stop=True,
                            )
                        ET = PW.tile([128, 1024], BF16, tag="ET", name=f"ET_{hp}_{jt}_{h01}")
                        nc.scalar.activation(ET[:], sp[:], AF.Exp, scale=float(D) ** -0.5)
                        # independent PE filler while ACT computes the exp
                        emitter.emit(3)
                        U = U0 if h01 == 0 else U1
                        for s in range(2):
                            ssl = slice(s * 512, (s + 1) * 512)
                            nc.tensor.matmul(
                                U[0 : D + 1, ssl], V4[:, jt, 2 * hp + h01, :], ET[:, ssl],
                                start=(jt == 0), stop=(jt == NT - 1),
                            )
                emitter.drain()
                # r = 1/denominator via exp(-ln(s)) on ACT (vector.reciprocal is
                # ~8 cyc/elem iterative divide -- far slower), then partition-
                # broadcast on GpSimd (otherwise idle)
                rr = PR.tile([1, N], F32, tag="row", name=f"rr_{hp}")
                nc.scalar.activation(rr[0:1, 0:1024], U0[D : D + 1, :], AF.Ln)
                nc.scalar.activation(rr[0:1, 1024:2048], U1[D : D + 1, :], AF.Ln)
                rrb = PW.tile([1, N], BF16, tag="rrb", name=f"rrb_{hp}")
                nc.scalar.activation(rrb[:], rr[:], AF.Exp, scale=-1.0)
                rbB = PW.tile([64, N], BF16, tag="rbB")
                nc.gpsimd.partition_broadcast(rbB[:], rrb[:])
                nc.vector.tensor_tensor(
                    AO[0:64, hp, :], U0[0:D, :], rbB[:, 0:1024], ALU.mult
                )
                AOtmp = PW.tile([64, NQ], BF16, tag="AOtmp", name=f"AOtmp_{hp}")
                nc.vector.tensor_tensor(
                    AOtmp[:], U1[0:D, :], rbB[:, 1024:2048], ALU.mult
                )
                nc.sync.dma_start(AO[64:128, hp, :], AOtmp[:])

            # ---- phase E: output projection + bias ----
            for ot in range(CT):
                osl = slice(ot * 128, (ot + 1) * 128)
                po = PS.tile([128, 1024], F32, tag="ps", name=f"po_{ot}")
                for s in range(2):
                    ssl = slice(s * 512, (s + 1) * 512)
                    nc.tensor.matmul(
                        po[:, ssl], wob[:, osl], onesI[:, ssl],
                        start=True, stop=False,
                    )
                    for ft in range(CT):
                        nc.tensor.matmul(
                            po[:, ssl], WO[:, ft, osl], AO[:, ft, ssl],
                            start=False, stop=(ft == CT - 1),
                        )
                outsb = PW.tile([128, 1024], F32, tag="outsb")
                nc.vector.tensor_copy(outsb[:], po[:])
                nc.sync.dma_start(outT[osl, :], outsb[:])

    nc.finalize()
    return nc


def _get_nc():
    global _NC
    if _NC is None:
        import os
        _NC = build(debug=os.environ.get("KDEBUG", "0") == "1")
    return _NC


def kernel(x, ln_w, ln_b, w_qkv, w_out, b_out):
    global LAST
    x = np.asarray(x, dtype=np.float32)
    ln_w = np.asarray(ln_w, dtype=np.float32)
    ln_b = np.asarray(ln_b, dtype=np.float32)
    w_qkv = np.asarray(w_qkv, dtype=np.float32)
    w_out = np.asarray(w_out, dtype=np.float32)
    b_out = np.asarray(b_out, dtype=np.float32)

    bf16 = ml_dtypes.bfloat16
    # W'' = [ (w_qkv * ln_w)^T ; rowsum of (w_qkv*ln_w) ; w_qkv @ ln_b ]
    wprime = w_qkv * ln_w[None, :]
    wqkvT = np.concatenate(
        [wprime.T, wprime.sum(axis=1)[None, :], (w_qkv @ ln_b)[None, :]], axis=0
    ).astype(bf16)
    woutT = np.concatenate([w_out.T, b_out[None, :]], axis=0).astype(bf16)

    in_maps = []
    for c in range(8):
        b, g = c // 2, c % 2
        order = np.r_[g * NQ : (g + 1) * NQ, (1 - g) * NQ : (2 - g) * NQ]
        xTc = np.ascontiguousarray(x[b][order].T)
        in_maps.append({"xT": xTc, "wqkvT": wqkvT, "woutT": woutT})

    nc = _get_nc()
    LAST = run_bass_kernel_spmd(nc, in_maps, core_ids=list(range(8)))

    out = np.empty((B, N, DIM), dtype=np.float32)
    for c in range(8):
        b, g = c // 2, c % 2
        out[b, g * NQ : (g + 1) * NQ, :] = LAST.results[c]["outT"].T
    return out



# revision 3
# speedup vs baseline: 1.2188x; 1.2188x over previous
"""Fused LayerNorm + multi-head attention Trainium2 kernel, 8-core SPMD.

Problem: x[4, 2048, 768] -> LN -> QKV (w_qkv[2304, 768]) -> 12-head attention
         -> out proj (w_out[768, 768] + b_out). f32 I/O, bf16 tensor-engine compute.

Sharding: core c handles batch b=c//2, query-half g=c%2 (1024 queries each).
Each core receives the FULL (rotated) sequence of its batch so K/V are computed
locally -- no collectives. The token order is rotated per-core so the core's own
query chunk is always columns [0, 1024) => identical SPMD program on all cores.

v2 design notes (vs baseline):
- LayerNorm is folded entirely into the QKV matmul operating on RAW bf16 x:
  qkv = rstd[n] * (W'x - mu[n]*C + rstdinv[n]*D) with W' = (w_qkv*ln_w)^T,
  C = rowsum(W'), D = w_qkv@ln_b.  The -mu and rstdinv per-token rows are two
  appended contraction rows; the rstd[n] factor is applied by the DVE during
  PSUM evacuation (fused with the f32->bf16 cast that was needed anyway).
  This removes the x~ staging pass and the PE stall behind it.
- Scores matmuls for the two heads of a pair are issued back-to-back on
  disjoint PE row groups (contraction=64: partitions 0:64 vs 64:128), so the
  hardware runs them concurrently (~2x on the scores phase).
- One 4-slot PSUM pool ([128,1024] = 2 banks each = all 8 banks): per head
  pair, U0/U1 hold two slots for the whole pass while score tiles ping-pong
  through the other two; K/Q units for the NEXT pair reuse freed slots in the
  inter-pass gap.  Keeps the PE dense so the HAM clock gate stays at 2.4 GHz.
"""

import numpy as np
import ml_dtypes

import concourse.bass as bass
import concourse.tile as tile
from concourse import bacc, mybir
from concourse.bass_utils import run_bass_kernel_spmd

F32 = mybir.dt.float32
BF16 = mybir.dt.bfloat16
AF = mybir.ActivationFunctionType
ALU = mybir.AluOpType

DIM = 768
HEADS = 12
B, N = 4, 2048
D = 64          # head dim
NQ = 1024       # queries per core
CT = 6          # 768 / 128 channel tiles
NT = 16         # 2048 / 128 token tiles
HP = 6          # head pairs

LAST = None  # BassKernelResults of the most recent run (for test harness)
_NC = None


def build(debug=False):
    nc = bacc.Bacc("TRN2", target_bir_lowering=False, debug=False, num_devices=8)

    xT = nc.dram_tensor("xT", [DIM, N], F32, kind="ExternalInput")
    wqkvT = nc.dram_tensor("wqkvT", [DIM + 2, 3 * DIM], BF16, kind="ExternalInput")
    woutT = nc.dram_tensor("woutT", [DIM + 1, DIM], BF16, kind="ExternalInput")
    outT = nc.dram_tensor("outT", [DIM, NQ], F32, kind="ExternalOutput")

    with tile.TileContext(nc) as tc:
        with (
            tc.tile_pool(name="persist", bufs=1) as P1,
            tc.tile_pool(name="work", bufs=2) as PW,
            tc.tile_pool(name="et", bufs=3) as PE_,
            tc.tile_pool(name="rows", bufs=2) as PR,
            tc.tile_pool(name="ps", bufs=4, space="PSUM") as PS,
            tc.tile_pool(name="dram", bufs=1, space="DRAM") as PD,
        ):
            # ---- persistent SBUF tensors ----
            xb16 = P1.tile([128, CT, N], BF16, tag="xb16")   # raw x, bf16
            xex = P1.tile([2, N], BF16)                      # rows: [-mu ; 1/rstd]
            wq = P1.tile([128, CT, 3 * DIM], BF16)           # W'' rows 0..767
            wex = P1.tile([2, 3 * DIM], BF16)                # W'' rows 768..769 (C; D)
            WO = P1.tile([128, CT, DIM], BF16)               # w_out^T  (f-major tiles)
            wob = P1.tile([1, DIM], BF16)                    # b_out row
            KT = P1.tile([128, CT, N], BF16)                 # K^T channel-major
            QT = P1.tile([128, CT, NQ], BF16)                # Q^T channel-major
            V4 = P1.tile([128, NT, HEADS, D + 1], BF16)      # V token-major + ones col
            AO = P1.tile([128, CT, NQ], BF16)                # attention out (f-major)
            rstdB = P1.tile([128, N], BF16)                  # rstd bcast to 128 parts
            rstd_col = P1.tile([128, NT], F32)               # rstd token-in-partition
            ones1b = P1.tile([128, 1], BF16)                 # bf16 ones column
            onesr = P1.tile([1, 128], F32)                   # f32 ones row (bcast lhsT)
            onesI = P1.tile([1, NQ], BF16)                   # bf16 ones row (bias rhs)
            epsc = P1.tile([1, 1], F32)

            nc.vector.memset(epsc[:], 1e-5)
            nc.vector.memset(ones1b[:], 1.0)
            nc.vector.memset(onesr[:], 1.0)
            nc.vector.memset(onesI[:], 1.0)
            nc.vector.memset(V4[:, :, :, D : D + 1], 1.0)

            # ---- weight DMAs ----
            for ct in range(CT):
                nc.sync.dma_start(wq[:, ct, :], wqkvT[ct * 128 : (ct + 1) * 128, :])
                nc.sync.dma_start(WO[:, ct, :], woutT[ct * 128 : (ct + 1) * 128, :])
            nc.sync.dma_start(wex[:], wqkvT[DIM : DIM + 2, :])
            nc.sync.dma_start(wob[:], woutT[DIM : DIM + 1, :])

            # ---- phase A: x load + cast + token stats via ones-matmuls ----
            # stats rows in PSUM: row 0 = sum, row 64 = sumsq, row 32 = mu^2 scratch
            sts = [PS.tile([128, 1024], F32, tag="ps", name=f"st_{h}") for h in range(2)]
            for ct in range(CT):
                for h in range(2):
                    hsl = slice(h * 1024, (h + 1) * 1024)
                    xin = PW.tile([128, 1024], F32, tag="f32w", name=f"xin_{ct}_{h}")
                    nc.sync.dma_start(xin[:], xT[ct * 128 : (ct + 1) * 128, hsl])
                    nc.vector.tensor_copy(xb16[:, ct, hsl], xin[:])
                    xsq = PE_.tile([128, 1024], BF16, tag="et", name=f"xsq_{ct}_{h}")
                    nc.vector.tensor_tensor(
                        xsq[:], xb16[:, ct, hsl], xb16[:, ct, hsl], ALU.mult
                    )
                    st = sts[h]
                    for s in range(2):
                        osl = slice(s * 512, (s + 1) * 512)
                        xsl = slice(h * 1024 + s * 512, h * 1024 + s * 512 + 512)
                        nc.tensor.matmul(
                            st[0:1, osl], ones1b[:], xb16[:, ct, xsl],
                            start=(ct == 0), stop=(ct == CT - 1),
                        )
                        nc.tensor.matmul(
                            st[64:65, osl], ones1b[:], xsq[:, osl],
                            start=(ct == 0), stop=(ct == CT - 1),
                        )

            # ---- rows: mu, var -> rstd (f32), 1/rstd + -mu (bf16 x-extra rows) ----
            r_var = PR.tile([1, N], F32, tag="row", name="r_var")
            r_mu = PR.tile([1, N], F32, tag="row", name="r_mu")
            for h in range(2):
                hsl = slice(h * 1024, (h + 1) * 1024)
                nc.vector.tensor_scalar_mul(r_var[:, hsl], sts[h][64:65, :], 1.0 / DIM)
                nc.vector.tensor_scalar_mul(r_mu[:, hsl], sts[h][0:1, :], 1.0 / DIM)
                nc.vector.tensor_tensor(
                    sts[h][32:33, :], r_mu[:, hsl], r_mu[:, hsl], ALU.mult
                )
                nc.vector.tensor_tensor(
                    r_var[:, hsl], r_var[:, hsl], sts[h][32:33, :], ALU.subtract
                )
            # ln(var+eps); rstdinv = exp(+0.5 ln); rstd = exp(-0.5 ln)
            nc.scalar.activation(r_var[:], r_var[:], AF.Ln, bias=epsc[:])
            # engines can only write at 32-aligned partition bases, so stage
            # the rstdinv row at partition 0 and DMA-shift it into xex row 1
            rinv = PW.tile([1, N], BF16, tag="rrb", name="rinv")
            nc.scalar.activation(rinv[:], r_var[:], AF.Exp, scale=0.5)
            nc.sync.dma_start(xex[1:2, :], rinv[:])
            nc.scalar.activation(r_var[:], r_var[:], AF.Exp, scale=-0.5)
            nc.vector.tensor_scalar_mul(xex[0:1, :], r_mu[:], -1.0)

            # rstd broadcast to all 128 partitions (for K/Q postscale)
            psb = [PS.tile([128, 1024], F32, tag="ps", name=f"rb_{h}") for h in range(2)]
            for jc in range(4):
                sl = slice(jc * 512, (jc + 1) * 512)
                nc.tensor.matmul(
                    psb[jc // 2][:, (jc % 2) * 512 : (jc % 2 + 1) * 512],
                    onesr[:], r_var[:, sl],
                )
            for h in range(2):
                nc.vector.tensor_copy(rstdB[:, h * 1024 : (h + 1) * 1024], psb[h][:])
            # rstd in token-in-partition layout (for V postscale), via DRAM bounce
            dscr = PD.tile([1, N], F32, name="dscr")
            nc.sync.dma_start(dscr[:], r_var[:])
            nc.sync.dma_start(
                rstd_col[:], dscr[0:1, :].rearrange("o (t p) -> (o p) t", p=128)
            )

            # ---- QKV unit emitters (raw x matmul + fused postscale) ----
            def kq_unit(which, hp, nh):
                base = DIM if which == "K" else 0
                fsl = slice(base + hp * 128, base + (hp + 1) * 128)
                acc = PS.tile(
                    [128, 1024], F32, tag="ps", name=f"kq{which}_{hp}_{nh}"
                )
                for s in range(2):
                    psl = slice(s * 512, (s + 1) * 512)
                    nsl = slice(nh * 1024 + s * 512, nh * 1024 + s * 512 + 512)
                    for ct in range(CT + 1):
                        nc.tensor.matmul(
                            acc[:, psl],
                            wq[:, ct, fsl] if ct < CT else wex[:, fsl],
                            xb16[:, ct, nsl] if ct < CT else xex[:, nsl],
                            start=(ct == 0), stop=(ct == CT),
                        )
                dst = KT if which == "K" else QT
                nsl = slice(nh * 1024, (nh + 1) * 1024)
                nc.vector.tensor_tensor(
                    dst[:, hp, nsl], acc[:], rstdB[:, nsl], ALU.mult
                )

            def emit_kq(hp):
                kq_unit("Q", hp, 0)
                kq_unit("K", hp, 0)
                kq_unit("K", hp, 1)

            def v_unit(nt):
                nsl = slice(nt * 128, (nt + 1) * 128)
                acc = PS.tile([128, 1024], F32, tag="ps", name=f"v_{nt}")
                for ct in range(CT + 1):
                    # matmul output must not straddle a 2KB PSUM bank: 768 = 512+256
                    for lo, sz in ((0, 512), (512, 256)):
                        fsl = slice(2 * DIM + lo, 2 * DIM + lo + sz)
                        nc.tensor.matmul(
                            acc[:, lo : lo + sz],
                            xb16[:, ct, nsl] if ct < CT else xex[:, nsl],
                            wq[:, ct, fsl] if ct < CT else wex[:, fsl],
                            start=(ct == 0), stop=(ct == CT),
                        )
                nc.vector.tensor_scalar(
                    V4[:, nt, :, 0:D],
                    acc[:, 0:DIM].rearrange("p (h d) -> p h d", h=HEADS),
                    rstd_col[:, nt : nt + 1],
                    None,
                    ALU.mult,
                )

            for nt in range(NT):
                v_unit(nt)
            emit_kq(0)

            # ---- attention, one head pair per pass ----
            for hp in range(HP):
                U0 = PS.tile([128, 1024], F32, tag="ps", name=f"U0_{hp}")
                U1 = PS.tile([128, 1024], F32, tag="ps", name=f"U1_{hp}")
                for jt in range(NT):
                    jsl = slice(jt * 128, (jt + 1) * 128)
                    sp0 = PS.tile([128, 1024], F32, tag="ps", name=f"sp0_{hp}_{jt}")
                    sp1 = PS.tile([128, 1024], F32, tag="ps", name=f"sp1_{hp}_{jt}")
                    # scores: two heads on disjoint PE row groups, issued
                    # adjacently so they run concurrently (row tiling)
                    for s in range(2):
                        ssl = slice(s * 512, (s + 1) * 512)
                        nc.tensor.matmul(
                            sp0[:, ssl], KT[0:64, hp, jsl], QT[0:64, hp, ssl],
                            start=True, stop=True,
                        )
                        nc.tensor.matmul(
                            sp1[:, ssl], KT[64:128, hp, jsl], QT[64:128, hp, ssl],
                            start=True, stop=True,
                        )
                    ET0 = PE_.tile([128, 1024], BF16, tag="et", name=f"ET0_{hp}_{jt}")
                    ET1 = PE_.tile([128, 1024], BF16, tag="et", name=f"ET1_{hp}_{jt}")
                    nc.scalar.activation(ET0[:], sp0[:], AF.Exp, scale=float(D) ** -0.5)
                    nc.scalar.activation(ET1[:], sp1[:], AF.Exp, scale=float(D) ** -0.5)
                    for s in range(2):
                        ssl = slice(s * 512, (s + 1) * 512)
                        nc.tensor.matmul(
                            U0[0 : D + 1, ssl], V4[:, jt, 2 * hp, :], ET0[:, ssl],
                            start=(jt == 0), stop=(jt == NT - 1),
                        )
                        nc.tensor.matmul(
                            U1[0 : D + 1, ssl], V4[:, jt, 2 * hp + 1, :], ET1[:, ssl],
                            start=(jt == 0), stop=(jt == NT - 1),
                        )
                # softmax denominators: r = 1/den via exp(-ln(den)) on ACT,
                # partition-broadcast on GpSimd (otherwise idle)
                rr = PR.tile([1, N], F32, tag="row", name=f"rr_{hp}")
                nc.scalar.activation(rr[0:1, 0:1024], U0[D : D + 1, :], AF.Ln)
                nc.scalar.activation(rr[0:1, 1024:2048], U1[D : D + 1, :], AF.Ln)
                rrb = PW.tile([1, N], BF16, tag="rrb", name=f"rrb_{hp}")
                nc.scalar.activation(rrb[:], rr[:], AF.Exp, scale=-1.0)
                rbB = PW.tile([64, N], BF16, tag="rbB")
                nc.gpsimd.partition_broadcast(rbB[:], rrb[:])
                nc.vector.tensor_tensor(
                    AO[0:64, hp, :], U0[0:D, :], rbB[:, 0:1024], ALU.mult
                )
                AOtmp = PW.tile([64, NQ], BF16, tag="AOtmp", name=f"AOtmp_{hp}")
                nc.vector.tensor_tensor(
                    AOtmp[:], U1[0:D, :], rbB[:, 1024:2048], ALU.mult
                )
                nc.sync.dma_start(AO[64:128, hp, :], AOtmp[:])
                if hp + 1 < HP:
                    emit_kq(hp + 1)

            # ---- output projection + bias ----
            for ot in range(CT):
                osl = slice(ot * 128, (ot + 1) * 128)
                po = PS.tile([128, 1024], F32, tag="ps", name=f"po_{ot}")
                for s in range(2):
                    ssl = slice(s * 512, (s + 1) * 512)
                    nc.tensor.matmul(
                        po[:, ssl], wob[:, osl], onesI[:, ssl],
                        start=True, stop=False,
                    )
                    for ft in range(CT):
                        nc.tensor.matmul(
                            po[:, ssl], WO[:, ft, osl], AO[:, ft, ssl],
                            start=False, stop=(ft == CT - 1),
                        )
                outsb = PW.tile([128, 1024], F32, tag="f32w", name=f"outsb_{ot}")
                nc.vector.tensor_copy(outsb[:], po[:])
                nc.sync.dma_start(outT[osl, :], outsb[:])

    nc.finalize()
    return nc


def _get_nc():
    global _NC
    if _NC is None:
        _NC = build()
    return _NC


def kernel(x, ln_w, ln_b, w_qkv, w_out, b_out):
    global LAST
    x = np.asarray(x, dtype=np.float32)
    ln_w = np.asarray(ln_w, dtype=np.float32)
    ln_b = np.asarray(ln_b, dtype=np.float32)
    w_qkv = np.asarray(w_qkv, dtype=np.float32)
    w_out = np.asarray(w_out, dtype=np.float32)
    b_out = np.asarray(b_out, dtype=np.float32)

    bf16 = ml_dtypes.bfloat16
    # W'' = [ (w_qkv * ln_w)^T ; rowsum of (w_qkv*ln_w) ; w_qkv @ ln_b ]
    wprime = w_qkv * ln_w[None, :]
    wqkvT = np.concatenate(
        [wprime.T, wprime.sum(axis=1)[None, :], (w_qkv @ ln_b)[None, :]], axis=0
    ).astype(bf16)
    woutT = np.concatenate([w_out.T, b_out[None, :]], axis=0).astype(bf16)

    in_maps = []
    for c in range(8):
        b, g = c // 2, c % 2
        order = np.r_[g * NQ : (g + 1) * NQ, (1 - g) * NQ : (2 - g) * NQ]
        xTc = np.ascontiguousarray(x[b][order].T)
        in_maps.append({"xT": xTc, "wqkvT": wqkvT, "woutT": woutT})

    nc = _get_nc()
    LAST = run_bass_kernel_spmd(nc, in_maps, core_ids=list(range(8)))

    out = np.empty((B, N, DIM), dtype=np.float32)
    for c in range(8):
        b, g = c // 2, c % 2
        out[b, g * NQ : (g + 1) * NQ, :] = LAST.results[c]["outT"].T
    return out
